# revision 1
# baseline (speedup 1.0000x reference)
"""GAT (nn_GAT_1726576853727) Trainium2 Bass kernel, 8-core SPMD.

Math (per head h, graph b):
  Wh = x[b] @ W[h,b]                                  [14, 1024]
  Wh1 = Wh @ a1[h,b], Wh2 = Wh @ a2[h,b]              [14]
  e[n,m] = leaky_relu(Wh1[n] + Wh2[m], 0.2)
  att[:,m] = softmax_n(where(adj[b] > 0, e, -9e15))   (normalize over n)
  hp[n,:] = sum_m att[n,m] Wh[m,:]  -> flatten to [14*1024]
  out_h[b] = hp @ fc_w[h].T + fc_b[h]                 [1024]
  out = log_softmax(sum_h out_h, axis=-1)             [32, 1024]

Sharding: core c -> head h=c//2, batch half c%2 (16 graphs each).
Head-sum via AllReduce over {0,2,4,6} and {1,3,5,7}; log_softmax on
device; host concatenates core0 rows 0:16 with core1 rows 16:32.

Layout: graphs are processed in groups of <=3, each graph in a
32-partition slot at base 0/32/64 (PE tile_position requires 32-aligned bases).  The
attention math is kept transposed (eT[m,n]) so the softmax is a
free-axis reduction; e is built with K=2 outer-sum matmuls; h_prime is
produced directly transposed (hpT[f,b]) so the fc contraction reads
host-pretransposed fc_w.T tiles in natural row-major layout.
"""

import os
import sys

sys.path.insert(0, "/opt/trn_rl_repo")
os.environ.setdefault("NEURON_RT_RESET_CORES", "1")

import numpy as np

B, N, IN_F, OUT_F, H = 32, 14, 1024, 1024, 4
ALPHA, NEG = 0.2, -9e15
N_CORES = 8
B_LOC = B // 2                      # graphs per core
SLOT = 32                           # PE tile_position: bases must be 0/32/64
GROUP_SIZES = [3, 3, 3, 3, 3, 1]    # graphs per group (slots at 0/32/64)
GROWS = 96                          # partition rows used per group (3 slots)
GROUP_STARTS = [0, 3, 6, 9, 12, 15]
NG = len(GROUP_SIZES)
NT = N * OUT_F // 128               # 112 f-tiles of 128 for the fc contraction
MM_F32R = True                      # PE fast-fp32 mode for the two big matmuls
OSPLIT = True                       # split fc over output halves + hp exchange

_CACHE = {}


def _build_nc(f32r: bool, variant: str = "full", reps: int = 1,
              osplit: bool = False):
    import concourse.bacc as bacc
    import concourse.mybir as mybir
    import concourse.tile as tile

    f32 = mybir.dt.float32
    mm_dt = mybir.dt.float32r if f32r else f32

    def mm(ap):
        return ap

    nc = bacc.Bacc("TRN2", target_bir_lowering=False, debug=False,
                   num_devices=N_CORES)

    OH = OUT_F // 2 if osplit else OUT_F      # fc output slice per core
    OROWS = B if osplit else B_LOC            # rows of the final output

    xT = nc.dram_tensor("xT", [IN_F, B_LOC * N], mm_dt, kind="ExternalInput").ap()
    Wc = nc.dram_tensor("Wc", [B_LOC, IN_F, OUT_F], mm_dt, kind="ExternalInput").ap()
    a12p = nc.dram_tensor("a12p", [2 * NG * GROWS, OUT_F], f32,
                          kind="ExternalInput").ap()
    adjp = nc.dram_tensor("adjp", [NG * GROWS, N], f32, kind="ExternalInput").ap()
    fcwT = nc.dram_tensor("fcwT", [N * OUT_F, OH], mm_dt, kind="ExternalInput").ap()
    fcb = nc.dram_tensor("fcb", [1, OH], f32, kind="ExternalInput").ap()
    eye = nc.dram_tensor("eye", [128, 128], f32, kind="ExternalInput").ap()
    out = nc.dram_tensor("out", [OROWS, OUT_F], f32, kind="ExternalOutput").ap()

    with tile.TileContext(nc) as tc:
        with (
            tc.tile_pool(name="const", bufs=1) as cpool,
            tc.tile_pool(name="wstream", bufs=3) as wpool,
            tc.tile_pool(name="fcwstream", bufs=3) as fcwpool,
            tc.tile_pool(name="whsb", bufs=2) as whsbpool,
            tc.tile_pool(name="attn", bufs=2) as apool,
            tc.tile_pool(name="psum_wh", bufs=1, space="PSUM") as ps_wh,
            tc.tile_pool(name="psum_small", bufs=1, space="PSUM") as ps_sm,
            tc.tile_pool(name="psum_hp", bufs=2, space="PSUM") as ps_hp,
            tc.tile_pool(name="psum_fc", bufs=1, space="PSUM") as ps_fc,
            tc.tile_pool(name="dram", bufs=1, space="DRAM") as dpool,
        ):
          for _rep in range(reps):
              # ---- resident inputs -------------------------------------------
              xT_sb = cpool.tile([128, 8, B_LOC * N], mm_dt, tag="xT")
              nc.sync.dma_start(out=xT_sb[:],
                                in_=xT.rearrange("(k p) t -> p k t", p=128))
              eye_sb = cpool.tile([128, 128], f32, tag="eye")
              nc.sync.dma_start(out=eye_sb[:], in_=eye[:])
              fcb_sb = cpool.tile([1, OH], f32, tag="fcb")
              nc.sync.dma_start(out=fcb_sb[:], in_=fcb[:])
              ones_sb = cpool.tile([1, B], f32, tag="ones")
              nc.vector.memset(ones_sb[:], 1.0)

              a_sb = {}
              adj_sb = {}
              for g in range(NG):
                  for j in range(2):  # 0 -> a1, 1 -> a2
                      t = cpool.tile([GROWS, OUT_F], f32, tag=f"a{j}g{g}",
                                     name=f"a{j}g{g}")
                      nc.sync.dma_start(
                          out=t[:],
                          in_=a12p[(j * NG + g) * GROWS:
                                   (j * NG + g + 1) * GROWS, :])
                      a_sb[(g, j)] = t
                  t = cpool.tile([GROWS, N], f32, tag=f"adjg{g}", name=f"adjg{g}")
                  nc.sync.dma_start(out=t[:],
                                    in_=adjp[g * GROWS:(g + 1) * GROWS, :])
                  adj_sb[g] = t

              hpT_sb = cpool.tile([128, NT * B_LOC], mm_dt, tag="hpT")

              # ---- phase 1: Wh + attention + hpT, grouped graphs -------------
              for g in range(NG):
                  wh_sb = whsbpool.tile([GROWS, OUT_F], f32, tag="whsb")
                  nc.vector.memset(wh_sb[:, :], 0.0)
                  for s in range(GROUP_SIZES[g]):
                      b = GROUP_STARTS[g] + s
                      wh_ps = ps_wh.tile([14, 1024], f32, tag="wh",
                                         name="wh_ps")
                      for kc in range(2):
                          w_t = wpool.tile([128, 4, OUT_F], mm_dt, tag="W")
                          nc.sync.dma_start(
                              out=w_t[:],
                              in_=Wc[b, kc * 512:(kc + 1) * 512, :]
                              .rearrange("(k p) o -> p k o", p=128))
                          for k4 in range(4):
                              k = kc * 4 + k4
                              for half in range(2):
                                  nc.tensor.matmul(
                                      wh_ps[:, half * 512:(half + 1) * 512],
                                      lhsT=xT_sb[:, k, b * N:(b + 1) * N],
                                      rhs=w_t[:, k4,
                                              half * 512:(half + 1) * 512],
                                      start=(k == 0), stop=(k == 7))
                      nc.vector.tensor_copy(
                          out=wh_sb[s * SLOT:s * SLOT + N, :],
                          in_=wh_ps[:, :])

                  if variant == "wh":
                      continue
                  # per-node dots -> cols [Wh2, 1, 1, Wh1]; transpose to rows
                  import concourse.mybir as _mb
                  cols = apool.tile([GROWS, 4], f32, tag="cols")
                  nc.vector.memset(cols[:, 1:3], 1.0)
                  scr = apool.tile([GROWS, OUT_F], f32, tag="scr", bufs=1)
                  nc.vector.tensor_tensor(out=scr[:], in0=wh_sb[:, :],
                                          in1=a_sb[(g, 1)][:, :],
                                          op=_mb.AluOpType.mult)
                  nc.vector.tensor_reduce(cols[:, 0:1], scr[:],
                                          _mb.AxisListType.X,
                                          _mb.AluOpType.add)
                  scr2g = apool.tile([GROWS, OUT_F], f32, tag="scr2g", bufs=1)
                  nc.vector.tensor_tensor(out=scr2g[:], in0=wh_sb[:, :],
                                          in1=a_sb[(g, 0)][:, :],
                                          op=_mb.AluOpType.mult)
                  nc.vector.tensor_reduce(cols[:, 3:4], scr2g[:],
                                          _mb.AxisListType.X,
                                          _mb.AluOpType.add)

                  if variant == "attn1":
                      continue
                  lhs_ps = ps_sm.tile([2, GROWS], f32, tag="small")
                  nc.tensor.transpose(lhs_ps[:], cols[:, 0:2], eye_sb[:GROWS, :GROWS])
                  rhs_ps = ps_sm.tile([2, GROWS], f32, tag="small")
                  nc.tensor.transpose(rhs_ps[:], cols[:, 2:4], eye_sb[:GROWS, :GROWS])
                  lhs_sb = apool.tile([2, GROWS], f32, tag="lhs_sb")
                  nc.vector.tensor_copy(out=lhs_sb[:], in_=lhs_ps[:])
                  rhs_sb = apool.tile([2, GROWS], f32, tag="rhs_sb")
                  nc.vector.tensor_copy(out=rhs_sb[:], in_=rhs_ps[:])

                  if variant == "attn2":
                      continue
                  # eT[m,n] = Wh2[m] + Wh1[n] via K=2 matmul per slot
                  e_ps = ps_sm.tile([GROWS, N], f32, tag="small")
                  nc.vector.memset(e_ps[:, :], 0.0)
                  for s in range(GROUP_SIZES[g]):
                      sl = slice(s * SLOT, s * SLOT + N)
                      nc.tensor.matmul(e_ps[sl, :], lhsT=lhs_sb[:, sl],
                                       rhs=rhs_sb[:, sl], start=True, stop=True)

                  e_sb = apool.tile([GROWS, N], f32, tag="e_sb")
                  nc.vector.tensor_copy(out=e_sb[:], in_=e_ps[:, :])
                  att = apool.tile([GROWS, N], f32, tag="att")
                  nc.vector.scalar_tensor_tensor(
                      att[:], e_sb[:], ALPHA, e_sb[:],
                      _mb.AluOpType.mult, _mb.AluOpType.max)
                  if variant == "attn3":
                      continue
                  mask = apool.tile([GROWS, N], mybir.dt.uint8, tag="mask")
                  nc.vector.tensor_scalar(mask[:], adj_sb[g][:, :], 0.0,
                                          None, _mb.AluOpType.is_gt)
                  msk = apool.tile([GROWS, N], f32, tag="msk")
                  nc.vector.memset(msk[:], NEG)
                  nc.vector.copy_predicated(msk[:], mask[:], att[:])
                  nmax = apool.tile([GROWS, 1], f32, tag="nmax")
                  nc.vector.tensor_reduce(nmax[:], msk[:], _mb.AxisListType.X,
                                          _mb.AluOpType.max, negate=True)
                  ssum = apool.tile([GROWS, 1], f32, tag="ssum")
                  nc.scalar.activation(att[:], msk[:],
                                       _mb.ActivationFunctionType.Exp,
                                       bias=nmax[:], scale=1.0,
                                       accum_out=ssum[:])
                  rcp = apool.tile([GROWS, 1], f32, tag="rcp")
                  nc.vector.reciprocal(rcp[:], ssum[:])
                  nc.vector.tensor_scalar_mul(att[:], att[:], rcp[:])

                  if variant == "attn":
                      continue
                  # hpT[oi, n] per graph -> packed [128, (n*8+c)*16 + b]
                  for s in range(GROUP_SIZES[g]):
                      b = GROUP_STARTS[g] + s
                      sl = slice(s * SLOT, s * SLOT + N)
                      hp_ps = ps_hp.tile([128, 8 * N], f32, tag="hp")
                      for c in range(8):
                          nc.tensor.matmul(
                              hp_ps[:, c * N:(c + 1) * N],
                              lhsT=wh_sb[sl, c * 128:(c + 1) * 128],
                              rhs=att[sl, :], start=True, stop=True)
                      dst = hpT_sb[:].rearrange("p (n c bb) -> p c n bb",
                                                n=N, c=8, bb=B_LOC)[:, :, :, b]
                      src = hp_ps[:].rearrange("p (c n) -> p c n", c=8, n=N)
                      nc.vector.tensor_copy(out=dst, in_=src)

              # ---- phase 2: fc over 112 f-tiles ------------------------------
              import concourse.mybir as _mb
              if variant == "wh":
                  nc.sync.dma_start(out=out[:], in_=wh_sb[0:B_LOC, :])
              elif variant == "attn1":
                  nc.sync.dma_start(out=out[0:B_LOC, 0:4], in_=cols[0:B_LOC, :])
              elif variant == "attn2":
                  nc.sync.dma_start(out=out[0:2, 0:128], in_=lhs_sb[:, :])
              elif variant == "attn3":
                  nc.sync.dma_start(out=out[0:B_LOC, 0:N], in_=att[0:B_LOC, :])
              elif variant == "attn":
                  nc.sync.dma_start(out=out[0:B_LOC, 0:N], in_=att[0:B_LOC, :])
              elif variant == "phase1":
                  nc.sync.dma_start(out=out[:], in_=hpT_sb[0:B_LOC, 0:OUT_F].bitcast(f32))
              elif osplit:
                  # exchange hpT within the head pair, fc on own o-half
                  hp_dram = dpool.tile([128, NT * B_LOC], mm_dt, name="hp_dram")
                  hp_all = dpool.tile([256, NT * B_LOC], mm_dt, name="hp_all")
                  nc.gpsimd.dma_start(out=hp_dram[:], in_=hpT_sb[:])
                  nc.gpsimd.collective_compute(
                      "AllGather", _mb.AluOpType.bypass,
                      replica_groups=[[0, 1], [2, 3], [4, 5], [6, 7]],
                      ins=[hp_dram.opt()], outs=[hp_all.opt()])
                  hp_all_sb = cpool.tile([128, 2 * NT * B_LOC], mm_dt,
                                         tag="hp_all")
                  nc.sync.dma_start(
                      out=hp_all_sb[:],
                      in_=hp_all[:].rearrange("(h p) (t bb) -> p t h bb",
                                              h=2, t=NT))
                  hp_view = hp_all_sb[:].rearrange(
                      "p (t h bb) -> p t (h bb)", t=NT, h=2)
                  fc_ps1 = ps_fc.tile([B, 512], f32, tag="fc0", name="fc_ps1")
                  for tc8 in range(NT // 8):
                      fcw_t = fcwpool.tile([128, 8, 512], mm_dt, tag="fcw")
                      nc.sync.dma_start(
                          out=fcw_t[:],
                          in_=fcwT[tc8 * 1024:(tc8 + 1) * 1024, :]
                          .rearrange("(t p) o -> p t o", p=128))
                      for t8 in range(8):
                          t = tc8 * 8 + t8
                          nc.tensor.matmul(
                              fc_ps1[:, :], lhsT=hp_view[:, t, :],
                              rhs=fcw_t[:, t8, :],
                              start=(t == 0), stop=False)
                  nc.tensor.matmul(fc_ps1[:, :], lhsT=ones_sb[:, :],
                                   rhs=fcb_sb[:, :], start=False, stop=True)
                  outh = cpool.tile([B, 512], f32, tag="outh")
                  nc.vector.tensor_copy(out=outh[:, :], in_=fc_ps1[:, :])

                  # head-sum over same-half cores, then pair-merge halves
                  cc_in = dpool.tile([B, 512], f32, name="cc_in")
                  cc_out = dpool.tile([B, 512], f32, name="cc_out")
                  nc.gpsimd.dma_start(out=cc_in[:], in_=outh[:, :])
                  nc.gpsimd.collective_compute(
                      "AllReduce", _mb.AluOpType.add,
                      replica_groups=[[0, 2, 4, 6], [1, 3, 5, 7]],
                      ins=[cc_in.opt()], outs=[cc_out.opt()])
                  half_all = dpool.tile([2 * B, 512], f32, name="half_all")
                  nc.gpsimd.collective_compute(
                      "AllGather", _mb.AluOpType.bypass,
                      replica_groups=[[0, 1], [2, 3], [4, 5], [6, 7]],
                      ins=[cc_out.opt()], outs=[half_all.opt()])
                  red = cpool.tile([B, OUT_F], f32, tag="red")
                  nc.gpsimd.dma_start(
                      out=red[:].rearrange("p (h o) -> p h o", h=2),
                      in_=half_all[:].rearrange("(h p) o -> p h o", h=2))

                  nmax2 = cpool.tile([B, 1], f32, tag="nmax2")
                  nc.vector.tensor_reduce(nmax2[:], red[:, :],
                                          _mb.AxisListType.X,
                                          _mb.AluOpType.max, negate=True)
                  scr2 = cpool.tile([B, OUT_F], f32, tag="scr2")
                  ssum2 = cpool.tile([B, 1], f32, tag="ssum2")
                  nc.scalar.activation(scr2[:], red[:, :],
                                       _mb.ActivationFunctionType.Exp,
                                       bias=nmax2[:], scale=1.0,
                                       accum_out=ssum2[:])
                  lns = cpool.tile([B, 1], f32, tag="lns")
                  nc.scalar.activation(lns[:], ssum2[:],
                                       _mb.ActivationFunctionType.Ln)
                  fin = cpool.tile([B, OUT_F], f32, tag="fin")
                  nc.vector.tensor_scalar(fin[:], red[:, :], nmax2[:], lns[:],
                                          _mb.AluOpType.add,
                                          _mb.AluOpType.subtract)
                  nc.sync.dma_start(out=out[:], in_=fin[:])
              else:
                  fc_ps = [ps_fc.tile([B_LOC, 512], f32, tag=f"fc{half}",
                                      name=f"fc_ps{half}")
                           for half in range(2)]
                  for tc4 in range(NT // 4):
                      fcw_t = fcwpool.tile([128, 4, OUT_F], mm_dt, tag="fcw")
                      nc.sync.dma_start(
                          out=fcw_t[:],
                          in_=fcwT[tc4 * 512:(tc4 + 1) * 512, :]
                          .rearrange("(t p) o -> p t o", p=128))
                      for t4 in range(4):
                          t = tc4 * 4 + t4
                          for half in range(2):
                              nc.tensor.matmul(
                                  fc_ps[half][:, :],
                                  lhsT=mm(hpT_sb[:, t * B_LOC:(t + 1) * B_LOC]),
                                  rhs=mm(fcw_t[:, t4,
                                               half * 512:(half + 1) * 512]),
                                  start=(t == 0), stop=False)
                  for half in range(2):
                      nc.tensor.matmul(fc_ps[half][:, :],
                                       lhsT=ones_sb[:, 0:B_LOC],
                                       rhs=fcb_sb[:, half * 512:(half + 1) * 512],
                                       start=False, stop=True)
                  outh = cpool.tile([B_LOC, OUT_F], f32, tag="outh")
                  for half in range(2):
                      nc.vector.tensor_copy(
                          out=outh[:, half * 512:(half + 1) * 512],
                          in_=fc_ps[half][:, :])

              # ---- head-sum AllReduce + log_softmax --------------------------
              if variant == "nocc" and not osplit:
                  nc.sync.dma_start(out=out[:], in_=outh[:, :])
              elif variant == "full" and not osplit:
                  cc_in = dpool.tile([B_LOC, OUT_F], f32)
                  cc_out = dpool.tile([B_LOC, OUT_F], f32)
                  nc.gpsimd.dma_start(out=cc_in[:], in_=outh[:, :])
                  nc.gpsimd.collective_compute(
                      "AllReduce", _mb.AluOpType.add,
                      replica_groups=[[0, 2, 4, 6], [1, 3, 5, 7]],
                      ins=[cc_in.opt()], outs=[cc_out.opt()])
                  red = cpool.tile([B_LOC, OUT_F], f32, tag="red")
                  nc.gpsimd.dma_start(out=red[:], in_=cc_out[:])

                  nmax2 = cpool.tile([B_LOC, 1], f32, tag="nmax2")
                  nc.vector.tensor_reduce(nmax2[:], red[:, :],
                                          _mb.AxisListType.X,
                                          _mb.AluOpType.max, negate=True)
                  scr2 = cpool.tile([B_LOC, OUT_F], f32, tag="scr2")
                  ssum2 = cpool.tile([B_LOC, 1], f32, tag="ssum2")
                  nc.scalar.activation(scr2[:], red[:, :],
                                       _mb.ActivationFunctionType.Exp,
                                       bias=nmax2[:], scale=1.0,
                                       accum_out=ssum2[:])
                  lns = cpool.tile([B_LOC, 1], f32, tag="lns")
                  nc.scalar.activation(lns[:], ssum2[:],
                                       _mb.ActivationFunctionType.Ln)
                  fin = cpool.tile([B_LOC, OUT_F], f32, tag="fin")
                  nc.vector.tensor_scalar(fin[:], red[:, :], nmax2[:], lns[:],
                                          _mb.AluOpType.add,
                                          _mb.AluOpType.subtract)
                  nc.sync.dma_start(out=out[:], in_=fin[:])

    nc.compile()
    return nc


def get_nc(f32r=MM_F32R, variant="full", reps=1, osplit=None):
    if osplit is None:
        osplit = OSPLIT
    key = ("nc", f32r, variant, reps, osplit)
    if key not in _CACHE:
        _CACHE[key] = _build_nc(f32r, variant, reps, osplit)
    return _CACHE[key]


def shard_inputs(x, adj, W, a, fc_w, fc_b, osplit=None):
    """Host-side layout prep: slice + transpose + slot-pad shards per core."""
    if osplit is None:
        osplit = OSPLIT
    x, adj, W, a = map(np.asarray, (x, adj, W, a))
    fc_w, fc_b = np.asarray(fc_w), np.asarray(fc_b)
    eye = np.eye(128, dtype=np.float32)
    fcwT = [np.ascontiguousarray(fc_w[h].T) for h in range(H)]
    maps = []
    for c in range(N_CORES):
        h, half = divmod(c, 2)
        bs = half * B_LOC
        xs = x[bs:bs + B_LOC]
        xTc = np.ascontiguousarray(xs.transpose(2, 0, 1)).reshape(IN_F,
                                                                  B_LOC * N)
        Wcc = np.ascontiguousarray(W[h, bs:bs + B_LOC])
        # slot-padded a1/a2 (zeros) and adj.T (-1), [2, NG, 128, .] layout
        a1v = a[h, bs:bs + B_LOC, :OUT_F, 0]
        a2v = a[h, bs:bs + B_LOC, OUT_F:, 0]
        adjv = adj[bs:bs + B_LOC].transpose(0, 2, 1)
        a12p = np.zeros((2, NG, GROWS, OUT_F), np.float32)
        adjp = np.full((NG, GROWS, N), -1.0, np.float32)
        for g in range(NG):
            for s in range(GROUP_SIZES[g]):
                b = GROUP_STARTS[g] + s
                a12p[0, g, s * SLOT:s * SLOT + N, :] = a1v[b]
                a12p[1, g, s * SLOT:s * SLOT + N, :] = a2v[b]
                adjp[g, s * SLOT:s * SLOT + N, :] = adjv[b]
        if osplit:
            o0 = half * (OUT_F // 2)
            fcw_c = np.ascontiguousarray(fcwT[h][:, o0:o0 + OUT_F // 2])
            fcb_c = np.ascontiguousarray(fc_b[h][None, o0:o0 + OUT_F // 2])
        else:
            fcw_c, fcb_c = fcwT[h], fc_b[h][None, :]
        maps.append({
            "xT": xTc, "Wc": Wcc,
            "a12p": a12p.reshape(2 * NG * GROWS, OUT_F),
            "adjp": adjp.reshape(NG * GROWS, N),
            "fcwT": fcw_c, "fcb": fcb_c, "eye": eye,
        })
    return maps


def kernel(x, adj, W, a, fc_w, fc_b):
    from concourse.bass_utils import run_bass_kernel_spmd

    nc = get_nc()
    in_maps = shard_inputs(x, adj, W, a, fc_w, fc_b)
    res = run_bass_kernel_spmd(nc, in_maps, core_ids=list(range(N_CORES)))
    if OSPLIT:
        return np.ascontiguousarray(res.results[0]["out"])
    top = res.results[0]["out"]
    bot = res.results[1]["out"]
    return np.concatenate([top, bot], axis=0)



# revision 8
# speedup vs baseline: 3.5445x; 3.5445x over previous
"""GAT (nn_GAT_1726576853727) Trainium2 Bass kernel, 8-core SPMD, bf16.

Math (per head h, graph b):
  Wh = x[b] @ W[h,b]                                  [14, 1024]
  Wh1 = Wh @ a1[h,b], Wh2 = Wh @ a2[h,b]              [14]
  e[n,m] = leaky_relu(Wh1[n] + Wh2[m], 0.2)
  att[:,m] = softmax_n(where(adj[b] > 0, e, -9e15))   (normalize over n)
  hp[n,:] = sum_m att[n,m] Wh[m,:]  -> flatten to [14*1024]
  out_h[b] = hp @ fc_w[h].T + fc_b[h]                 [1024]
  out = log_softmax(sum_h out_h, axis=-1)             [32, 1024]

Sharding: core c -> head h=c//2, batch half c%2 (16 graphs each).
fc is split over output halves (osplit) with an hpT AllGather inside
head pairs; head-sum via AllReduce over {0,2,4,6}/{1,3,5,7}; halves
merged with a pair AllGather; log_softmax on device; host returns
core 0's full [32, 1024] output.

Key layout/throughput choices vs the fp32 baseline:
  - x/W/fc_w/v12/hpT all bf16 (host-cast): halves HBM traffic; end-to-end
    rel err ~1.5e-3 vs the 2e-2 gate.
  - a-dots folded on host: v1=W@a1, v2=W@a2, so Wh1/Wh2 = x@v12 come from
    one small matmul stream (replaces the 4.7MB padded a12 broadcast input
    and the per-group mult+reduce+transpose pipeline).
  - graphs in groups of 4 at partition slots 0/32/64/96.
  - attention kept transposed (eT[m,n]) so softmax is a free-axis reduce;
    h_prime computed block-diagonally: 8 matmuls per group (not per graph).
  - W streamed one 2MiB DMA per graph; fcw tiles prefetched before the
    collective-dependent hp_all load so the DMA queue never drains.
"""

import os
import sys

sys.path.insert(0, "/opt/trn_rl_repo")
os.environ.setdefault("NEURON_RT_RESET_CORES", "1")

import numpy as np

B, N, IN_F, OUT_F, H = 32, 14, 1024, 1024, 4
ALPHA, NEG = 0.2, -9e15
N_CORES = 8
B_LOC = B // 2                      # graphs per core
SLOT = 32                           # PE tile_position: bases must be 0/32/64/96
GROUP_SIZES = [4, 4, 4, 4]
GROUP_STARTS = [0, 4, 8, 12]
GROWS = 128                         # partition rows per group (4 slots)
NG = len(GROUP_SIZES)
KC = IN_F // 128                    # 8 k-chunks of the contraction
OC = OUT_F // 128                   # 8 column chunks of Wh
NT = N * OUT_F // 128               # 112 f-tiles of 128 for the fc contraction
OH = OUT_F // 2                     # fc output slice per core (osplit)
TL = B_LOC * N                      # 224 node columns per core
MODE = "bf16"                       # "bf16" | "f32r"

_CACHE = {}


def _build_nc(mode: str = MODE, variant: str = "full", reps: int = 1):
    import concourse.bacc as bacc
    import concourse.mybir as mybir
    import concourse.tile as tile

    _mb = mybir
    f32 = mybir.dt.float32
    mm_dt = mybir.dt.bfloat16 if mode == "bf16" else mybir.dt.float32r

    nc = bacc.Bacc("TRN2", target_bir_lowering=False, debug=False,
                   num_devices=N_CORES)

    xTr = nc.dram_tensor("xTr", [128, KC * TL], mm_dt, kind="ExternalInput").ap()
    v12r = nc.dram_tensor("v12r", [128, KC * 2 * B_LOC], mm_dt,
                          kind="ExternalInput").ap()
    econst = nc.dram_tensor("econst", [2 * max(GROUP_SIZES), NG * GROWS],
                            mybir.dt.float32, kind="ExternalInput").ap()
    Wc = nc.dram_tensor("Wc", [B_LOC, IN_F, OUT_F], mm_dt,
                        kind="ExternalInput").ap()
    adjp = nc.dram_tensor("adjp", [NG * GROWS, N], f32, kind="ExternalInput").ap()
    fcwT = nc.dram_tensor("fcwT", [N * OUT_F, OH], mm_dt,
                          kind="ExternalInput").ap()
    fcb = nc.dram_tensor("fcb", [1, OH], f32, kind="ExternalInput").ap()
    out = nc.dram_tensor("out", [B, OUT_F], f32, kind="ExternalOutput").ap()

    with tile.TileContext(nc) as tc:
        with (
            tc.tile_pool(name="const", bufs=1) as cpool,
            tc.tile_pool(name="wstream", bufs=4) as wpool,
            tc.tile_pool(name="fcwstream", bufs=4) as fcwpool,
            tc.tile_pool(name="whsb", bufs=2) as whsbpool,
            tc.tile_pool(name="attn", bufs=2) as apool,
            tc.tile_pool(name="psum_v", bufs=1, space="PSUM") as ps_v,
            tc.tile_pool(name="psum_wha", bufs=2, space="PSUM") as ps_wha,
            tc.tile_pool(name="psum_whb", bufs=2, space="PSUM") as ps_whb,
            tc.tile_pool(name="psum_e", bufs=1, space="PSUM") as ps_e,
            tc.tile_pool(name="psum_hp", bufs=1, space="PSUM") as ps_hp,
            tc.tile_pool(name="psum_fc", bufs=1, space="PSUM") as ps_fc,
            tc.tile_pool(name="dram", bufs=1, space="DRAM") as dpool,
        ):
          for _rep in range(reps):
            # ---- resident inputs -------------------------------------------
            xT_sb = cpool.tile([128, KC, TL], mm_dt, tag="xT")
            nc.sync.dma_start(out=xT_sb[:],
                              in_=xTr.rearrange("p (k t) -> p k t", k=KC))
            v12_sb = cpool.tile([128, KC, 2 * B_LOC], mm_dt, tag="v12")
            nc.sync.dma_start(out=v12_sb[:],
                              in_=v12r.rearrange("p (k c) -> p k c", k=KC))
            fcb_sb = cpool.tile([1, OH], f32, tag="fcb")
            nc.sync.dma_start(out=fcb_sb[:], in_=fcb[:])
            ones_sb = cpool.tile([1, B], f32, tag="ones")
            nc.vector.memset(ones_sb[:], 1.0)
            adj_sb = {}
            for g in range(NG):
                t = cpool.tile([GROWS, N], f32, tag=f"adjg{g}", name=f"adjg{g}")
                nc.sync.dma_start(out=t[:],
                                  in_=adjp[g * GROWS:(g + 1) * GROWS, :])
                adj_sb[g] = t
            hpT_sb = cpool.tile([128, NT * B_LOC], mm_dt, tag="hpT")

            # ---- Wh1/Wh2 for every graph in one matmul stream --------------
            # vout[2b+0, b*N+n] = Wh2[b,n]; vout[2b+1, b*N+n] = Wh1[b,n]
            vout = ps_v.tile([2 * B_LOC, TL], f32, tag="vout", name="vout")
            for k in range(KC):
                nc.tensor.matmul(vout[:, :], lhsT=v12_sb[:, k, :],
                                 rhs=xT_sb[:, k, :],
                                 start=(k == 0), stop=(k == KC - 1))

            KE = 2 * max(GROUP_SIZES)
            if variant != "wh":
                # Stage Wh1/Wh2 in SBUF, then scatter them into the e-matmul
                # operands with small Pool-queue DMAs (engines cannot address
                # partition bases outside 0/32/64/96; DMA can).
                #   lhs_all[2s+0, g*GROWS+32s+m] = Wh2[b(g,s), m] (econst: 0)
                #   lhs_all[2s+1, g*GROWS+32s+m] = 1              (econst)
                #   rhs_all[2s+0, g*N+n] = 1 (memset)
                #   rhs_all[2s+1, g*N+n] = Wh1[b(g,s), n]
                # e_ps(g)[32s+m, n] = Wh2[m] + Wh1[n] per slot, one matmul.
                vsb = cpool.tile([2 * B_LOC, TL], f32, tag="vsb")
                nc.vector.tensor_copy(out=vsb[:], in_=vout[:, :])
                lhs_all = cpool.tile([KE, NG * GROWS], f32, tag="lhs_all")
                nc.sync.dma_start(out=lhs_all[:], in_=econst[:])
                rhs_all = cpool.tile([KE, NG * N], f32, tag="rhs_all")
                nc.vector.memset(rhs_all[:], 1.0)
                for g in range(NG):
                    for s in range(GROUP_SIZES[g]):
                        b = GROUP_STARTS[g] + s
                        nc.gpsimd.dma_start(
                            out=lhs_all[2 * s:2 * s + 1,
                                        g * GROWS + s * SLOT:
                                        g * GROWS + s * SLOT + N],
                            in_=vsb[2 * b:2 * b + 1, b * N:(b + 1) * N])
                        nc.gpsimd.dma_start(
                            out=rhs_all[2 * s + 1:2 * s + 2,
                                        g * N:g * N + N],
                            in_=vsb[2 * b + 1:2 * b + 2, b * N:(b + 1) * N])

            att_bd_last = None
            wh_last = None
            for g in range(NG):
                gs = GROUP_SIZES[g]
                g0 = GROUP_STARTS[g]

                # ---- attention (independent of the W stream) ---------------
                if variant != "wh":
                    e_ps = ps_e.tile([GROWS, N], f32, tag="e", name="e_ps")
                    nc.tensor.matmul(
                        e_ps[:, :],
                        lhsT=lhs_all[:, g * GROWS:(g + 1) * GROWS],
                        rhs=rhs_all[:, g * N:(g + 1) * N],
                        start=True, stop=True)
                    e_sb = apool.tile([GROWS, N], f32, tag="e_sb")
                    nc.vector.tensor_copy(out=e_sb[:], in_=e_ps[:, :])
                    att = apool.tile([GROWS, N], f32, tag="att")
                    nc.vector.scalar_tensor_tensor(
                        att[:], e_sb[:], ALPHA, e_sb[:],
                        _mb.AluOpType.mult, _mb.AluOpType.max)
                    mask = apool.tile([GROWS, N], mybir.dt.uint8, tag="mask")
                    nc.vector.tensor_scalar(mask[:], adj_sb[g][:, :], 0.0,
                                            None, _mb.AluOpType.is_gt)
                    msk = apool.tile([GROWS, N], f32, tag="msk")
                    nc.vector.memset(msk[:], NEG)
                    nc.vector.copy_predicated(msk[:], mask[:], att[:])
                    nmax = apool.tile([GROWS, 1], f32, tag="nmax")
                    nc.vector.tensor_reduce(nmax[:], msk[:], _mb.AxisListType.X,
                                            _mb.AluOpType.max, negate=True)
                    ssum = apool.tile([GROWS, 1], f32, tag="ssum")
                    nc.scalar.activation(att[:], msk[:],
                                         _mb.ActivationFunctionType.Exp,
                                         bias=nmax[:], scale=1.0,
                                         accum_out=ssum[:])
                    rcp = apool.tile([GROWS, 1], f32, tag="rcp")
                    nc.vector.reciprocal(rcp[:], ssum[:])
                    # block-diagonal attT: att_bd[32s+m, s*N+n] = attT[m,n]
                    att_bd = apool.tile([GROWS, gs * N], f32, tag="attbd")
                    nc.vector.memset(att_bd[:], 0.0)
                    for s in range(gs):
                        sl = slice(s * SLOT, s * SLOT + N)
                        nc.vector.tensor_scalar_mul(
                            att_bd[sl, s * N:(s + 1) * N], att[sl, :],
                            rcp[sl, :])
                    att_bd_last = att_bd
                if variant == "attn":
                    continue

                # ---- Wh matmul stream --------------------------------------
                wh_sb = whsbpool.tile([GROWS, OUT_F], f32, tag="whsb")
                wh_last = wh_sb
                nc.vector.memset(wh_sb[:, :], 0.0)
                for s in range(gs):
                    b = g0 + s
                    sl = slice(s * SLOT, s * SLOT + N)
                    w_t = wpool.tile([128, KC, OUT_F], mm_dt, tag="W")
                    nc.sync.dma_start(
                        out=w_t[:],
                        in_=Wc[b].rearrange("(k p) o -> p k o", p=128))
                    wh_a = ps_wha.tile([N, 512], f32, tag="wha", name="wh_a")
                    wh_b = ps_whb.tile([N, 512], f32, tag="whb", name="wh_b")
                    for k in range(KC):
                        nc.tensor.matmul(wh_a[:, :],
                                         lhsT=xT_sb[:, k, b * N:(b + 1) * N],
                                         rhs=w_t[:, k, 0:512],
                                         start=(k == 0), stop=(k == KC - 1))
                        nc.tensor.matmul(wh_b[:, :],
                                         lhsT=xT_sb[:, k, b * N:(b + 1) * N],
                                         rhs=w_t[:, k, 512:1024],
                                         start=(k == 0), stop=(k == KC - 1))
                    nc.vector.tensor_copy(out=wh_sb[sl, 0:512], in_=wh_a[:, :])
                    nc.scalar.copy(out=wh_sb[sl, 512:1024], in_=wh_b[:, :])
                if variant == "wh":
                    continue

                # ---- h_prime, transposed+packed: 8 matmuls per group -------
                hp_ps = ps_hp.tile([128, OC * gs * N], f32, tag="hp",
                                   name="hp_ps")
                for c in range(OC):
                    nc.tensor.matmul(hp_ps[:, c * gs * N:(c + 1) * gs * N],
                                     lhsT=wh_sb[:, c * 128:(c + 1) * 128],
                                     rhs=att_bd[:, :], start=True, stop=True)
                for s in range(gs):
                    b = g0 + s
                    dst = hpT_sb[:].rearrange("p (n c bb) -> p c n bb",
                                              n=N, c=OC, bb=B_LOC)[:, :, :, b]
                    src = hp_ps[:].rearrange("p (c s n) -> p s c n",
                                             c=OC, s=gs)[:, s, :, :]
                    nc.vector.tensor_copy(out=dst, in_=src)

            # ---- variant early outs ----------------------------------------
            if variant == "wh":
                nc.sync.dma_start(out=out[0:B, :], in_=wh_last[0:B, :])
                continue
            if variant == "attn":
                nc.sync.dma_start(out=out[0:B, 0:GROUP_SIZES[-1] * N],
                                  in_=att_bd_last[0:B, :])
                continue
            if variant == "phase1":
                nc.sync.dma_start(out=out[0:B, 0:NT * B_LOC // 2],
                                  in_=hpT_sb[0:B, :].bitcast(f32))
                continue

            # ---- phase 2: fc over 112 f-tiles ------------------------------
            def fcw_tile(tc8):
                t = fcwpool.tile([128, 8, OH], mm_dt, tag="fcw", name="fcw_t")
                nc.sync.dma_start(
                    out=t[:],
                    in_=fcwT[tc8 * 1024:(tc8 + 1) * 1024, :]
                    .rearrange("(t p) o -> p t o", p=128))
                return t

            if variant == "nocc":
                # fc on own batch half only, no collectives (attribution aid)
                hp_view = hpT_sb[:].rearrange("p (t bb) -> p t bb", t=NT)
                fc_ps = ps_fc.tile([B_LOC, OH], f32, tag="fc", name="fc_ps")
                for tc8 in range(NT // 8):
                    ft = fcw_tile(tc8)
                    for t8 in range(8):
                        tt = tc8 * 8 + t8
                        nc.tensor.matmul(fc_ps[:, :], lhsT=hp_view[:, tt, :],
                                         rhs=ft[:, t8, :],
                                         start=(tt == 0), stop=False)
                nc.tensor.matmul(fc_ps[:, :], lhsT=ones_sb[:, 0:B_LOC],
                                 rhs=fcb_sb[:, :], start=False, stop=True)
                outh = cpool.tile([B_LOC, OH], f32, tag="outh")
                nc.vector.tensor_copy(out=outh[:, :], in_=fc_ps[:, :])
                nc.sync.dma_start(out=out[0:B_LOC, 0:OH], in_=outh[:, :])
                continue

            # exchange hpT within the head pair; fc on own o-half for all 32
            hp_dram = dpool.tile([128, NT * B_LOC], mm_dt, name="hp_dram")
            hp_all = dpool.tile([256, NT * B_LOC], mm_dt, name="hp_all")
            nc.gpsimd.dma_start(out=hp_dram[:], in_=hpT_sb[:])
            nc.gpsimd.collective_compute(
                "AllGather", _mb.AluOpType.bypass,
                replica_groups=[[0, 1], [2, 3], [4, 5], [6, 7]],
                ins=[hp_dram.opt()], outs=[hp_all.opt()])
            # prefetch fcw tiles so DMA stays busy across the collective
            pre = [fcw_tile(0), fcw_tile(1), fcw_tile(2)]
            hp_all_sb = cpool.tile([128, 2, NT * B_LOC], mm_dt, tag="hp_all")
            nc.sync.dma_start(
                out=hp_all_sb[:],
                in_=hp_all[:].rearrange("(h p) f -> p h f", p=128))
            # interleave to [p, t, (h bb)] contiguous so each fc lhsT slice
            # is a single free dim (BIR allows only one on the weights AP)
            hp_comb = cpool.tile([128, NT, 2, B_LOC], mm_dt, tag="hp_comb")
            nc.vector.tensor_copy(
                out=hp_comb[:],
                in_=hp_all_sb[:].rearrange("p h (t bb) -> p t h bb", t=NT))
            hp_view = hp_comb[:].rearrange("p t h bb -> p t (h bb)")

            fc_ps = ps_fc.tile([B, OH], f32, tag="fc", name="fc_ps")
            for tc8 in range(NT // 8):
                ft = pre[tc8] if tc8 < 3 else fcw_tile(tc8)
                for t8 in range(8):
                    tt = tc8 * 8 + t8
                    nc.tensor.matmul(fc_ps[:, :], lhsT=hp_view[:, tt, :],
                                     rhs=ft[:, t8, :],
                                     start=(tt == 0), stop=False)
            nc.tensor.matmul(fc_ps[:, :], lhsT=ones_sb[:, :],
                             rhs=fcb_sb[:, :], start=False, stop=True)
            outh = cpool.tile([B, OH], f32, tag="outh")
            nc.vector.tensor_copy(out=outh[:, :], in_=fc_ps[:, :])

            # ---- head-sum AllReduce + pair merge + log_softmax -------------
            cc_in = dpool.tile([B, OH], f32, name="cc_in")
            cc_out = dpool.tile([B, OH], f32, name="cc_out")
            nc.gpsimd.dma_start(out=cc_in[:], in_=outh[:, :])
            nc.gpsimd.collective_compute(
                "AllReduce", _mb.AluOpType.add,
                replica_groups=[[0, 2, 4, 6], [1, 3, 5, 7]],
                ins=[cc_in.opt()], outs=[cc_out.opt()])
            half_all = dpool.tile([2 * B, OH], f32, name="half_all")
            nc.gpsimd.collective_compute(
                "AllGather", _mb.AluOpType.bypass,
                replica_groups=[[0, 1], [2, 3], [4, 5], [6, 7]],
                ins=[cc_out.opt()], outs=[half_all.opt()])
            red = cpool.tile([B, OUT_F], f32, tag="red")
            nc.sync.dma_start(
                out=red[:].rearrange("p (h o) -> p h o", h=2),
                in_=half_all[:].rearrange("(h p) o -> p h o", p=B))

            nmax2 = cpool.tile([B, 1], f32, tag="nmax2")
            nc.vector.tensor_reduce(nmax2[:], red[:, :], _mb.AxisListType.X,
                                    _mb.AluOpType.max, negate=True)
            scr2 = cpool.tile([B, OUT_F], f32, tag="scr2")
            ssum2 = cpool.tile([B, 1], f32, tag="ssum2")
            nc.scalar.activation(scr2[:], red[:, :],
                                 _mb.ActivationFunctionType.Exp,
                                 bias=nmax2[:], scale=1.0, accum_out=ssum2[:])
            lns = cpool.tile([B, 1], f32, tag="lns")
            nc.scalar.activation(lns[:], ssum2[:],
                                 _mb.ActivationFunctionType.Ln)
            fin = cpool.tile([B, OUT_F], f32, tag="fin")
            nc.vector.tensor_scalar(fin[:], red[:, :], nmax2[:], lns[:],
                                    _mb.AluOpType.add,
                                    _mb.AluOpType.subtract)
            nc.sync.dma_start(out=out[:], in_=fin[:])

    nc.compile()
    return nc


def get_nc(mode=MODE, variant="full", reps=1):
    key = ("nc", mode, variant, reps)
    if key not in _CACHE:
        _CACHE[key] = _build_nc(mode, variant, reps)
    return _CACHE[key]


def _np_dt(mode):
    import concourse.mybir as mybir
    return mybir.dt.np(mybir.dt.bfloat16 if mode == "bf16"
                       else mybir.dt.float32)


def shard_inputs(x, adj, W, a, fc_w, fc_b, mode=MODE):
    """Host-side prep: slice/transpose/cast per-core shards.

    Also folds the attention dot-products into the weights:
    v1 = W[h,b] @ a1[h,b], v2 = W[h,b] @ a2[h,b], so the device computes
    Wh1 = x @ v1 and Wh2 = x @ v2 directly.
    """
    np_dt = _np_dt(mode)
    x, adj, W, a = map(np.asarray, (x, adj, W, a))
    fc_w, fc_b = np.asarray(fc_w), np.asarray(fc_b)
    fcwT = [np.ascontiguousarray(fc_w[h].T) for h in range(H)]
    # static part of the e-matmul stationary operand: slot-mask ones rows
    KE = 2 * max(GROUP_SIZES)
    econst = np.zeros((KE, NG * GROWS), np.float32)
    for g in range(NG):
        for s in range(GROUP_SIZES[g]):
            econst[2 * s + 1, g * GROWS + s * SLOT:
                   g * GROWS + s * SLOT + N] = 1.0
    maps = []
    for c in range(N_CORES):
        h, half = divmod(c, 2)
        bs = half * B_LOC
        xs = x[bs:bs + B_LOC]                               # [16, 14, 1024]
        xT = xs.transpose(2, 0, 1).reshape(IN_F, TL)        # [1024, 224]
        xTr = np.ascontiguousarray(
            xT.reshape(KC, 128, TL).transpose(1, 0, 2).reshape(128, KC * TL)
        ).astype(np_dt)
        Ws = W[h, bs:bs + B_LOC]                            # [16, 1024, 1024] f32
        a1v = a[h, bs:bs + B_LOC, :OUT_F, 0]                # [16, 1024]
        a2v = a[h, bs:bs + B_LOC, OUT_F:, 0]
        v1 = np.matmul(Ws, a1v[:, :, None])[:, :, 0]        # [16, 1024] (i)
        v2 = np.matmul(Ws, a2v[:, :, None])[:, :, 0]
        v12 = np.empty((IN_F, 2 * B_LOC), np.float32)
        v12[:, 0::2] = v2.T                                 # col 2b+0 -> Wh2
        v12[:, 1::2] = v1.T                                 # col 2b+1 -> Wh1
        v12r = np.ascontiguousarray(
            v12.reshape(KC, 128, 2 * B_LOC).transpose(1, 0, 2)
            .reshape(128, KC * 2 * B_LOC)).astype(np_dt)
        Wcc = np.ascontiguousarray(Ws).astype(np_dt)
        adjv = adj[bs:bs + B_LOC].transpose(0, 2, 1)        # [16, m, n]
        adjp = np.full((NG, GROWS, N), -1.0, np.float32)
        for g in range(NG):
            for s in range(GROUP_SIZES[g]):
                b = GROUP_STARTS[g] + s
                adjp[g, s * SLOT:s * SLOT + N, :] = adjv[b]
        o0 = half * OH
        fcw_c = np.ascontiguousarray(fcwT[h][:, o0:o0 + OH]).astype(np_dt)
        fcb_c = np.ascontiguousarray(fc_b[h][None, o0:o0 + OH]).astype(
            np.float32)
        maps.append({
            "xTr": xTr, "v12r": v12r, "Wc": Wcc, "econst": econst,
            "adjp": adjp.reshape(NG * GROWS, N),
            "fcwT": fcw_c, "fcb": fcb_c,
        })
    return maps


def kernel(x, adj, W, a, fc_w, fc_b):
    from concourse.bass_utils import run_bass_kernel_spmd

    nc = get_nc()
    in_maps = shard_inputs(x, adj, W, a, fc_w, fc_b)
    res = run_bass_kernel_spmd(nc, in_maps, core_ids=list(range(N_CORES)))
    return np.ascontiguousarray(res.results[0]["out"])
